# revision 37
# baseline (speedup 1.0000x reference)
"""Distributed MultiHeadAttention kernel for 8 TRN2 NeuronCores.

Problem: B=4, S=2048, D=1024, H=16, DH=64, fp32 reference, full
(non-causal) attention. ~137 GFLOP total.

Sharding (head-parallel): core c owns batch b=c//2 and head-half hh=c%2
(8 heads = 4 head-pairs, all 2048 queries).  Q/K/V projections are
computed once globally (query-half sharding would duplicate K/V).  Each
core emits a PARTIAL output Y_c = (attn heads_hh) @ wo_hh [2048, 1024]
in bf16; the host sums the two partials per batch and adds the output
bias (the O-projection is linear over head groups), so no cross-core
communication is needed.  One SPMD program; per-core inputs differ only
in data (XT by batch, weight slices by head-half).

Per-core PE stream: 1568 matmuls x 512 moving columns = 803K columns
(~334 us at 2.4 GHz) vs 934K for query-half sharding.  The attention
inner loop is paced by the scalar engine's Exp ([128,1024] PSUM->SBUF
bf16, ~1.11 us per key chunk vs ~0.85 us of PE matmul), so the span is
roughly startup + the saturated exp stream + the endgame.

Schedule (measured on silicon, ~409.6 us vs 480 us for the v1
query-half-sharded kernel; abs-max rel err 2.9e-3):
- Upfront: V-projection for pairs 0-1 only (N=256 per-half split), Q0/K0.
  x arrives in column waves on 3 DMA issue queues; first matmul ~17 us.
- A fill queue interleaves deferred PE work (V23 projection, next pair's
  Q/K projections, first 8 output-projection token chunks) into the
  attention loop at 8 fixed kc slots per 512-query tile -- ~60% of the
  PE idle under the exp pacing.  CRITICAL: filling ALL the idle (or >8
  slots) raises sustained chip power enough that the package DVFS cuts
  the clock ~20% across every engine (exp 1.11->1.33 us), a net LOSS.
  The 8-slot density stays under the knee; leftovers drain as phased
  blocks at pair boundaries where the scalar is idle anyway.
- Softmax: scores for a head pair land in one [128,1024] psum (h0
  contracts on partitions 0-63, h1 on 64-127), ONE Exp per key chunk,
  no max-subtraction (|s| <= ~9, exp <= 6.5e3 fits fp16 P).  PV uses
  augmented stationary tiles ([V_h0|ones] and [ones|junk|V_h1]) so the
  softmax denominators accumulate in psum rows 64 / 0 for free.
- Tail per (pair, q2): 1/den = exp(-ln(den)) on a [65,512] sums-row
  span (both heads' rows copied to cols 0:512; ln/exp share one
  activation table; DVE reciprocal measured 6.5 ns/elem = 3.3 us per
  row and is NOT used), then a K=1 fp16 ones-matmul broadcasts the
  reciprocals and a DVE multiply (reading psum directly) writes aot
  fp16.  Pairs 0-2 defer ENTIRE tails to their pair-end drain blocks
  (aot is only read in pair 3), keeping the scalar stream pure exps;
  pair 3's tails are slot-dispatched into the next q2 so the in-order
  PE queue never waits on them.
- Output projection: per 128-token chunk, 4 accumulating matmuls into
  psum, bf16 copy, DMA out; chunks 0-7 ride pair 3's fill slots, 8-11
  cover the final tail's latency, 12-15 follow it.
- walrus in this environment rejects >1 semaphore wait per instruction;
  a post-pass hoists extra waits onto standalone InstEventSemaphore.
"""
import numpy as np
import ml_dtypes
import concourse.bass as bass
import concourse.mybir as mybir
from concourse.tile import TileContext
from concourse.bass_utils import run_bass_kernel_spmd


def _ensure_trace_shim():
    """concourse's axon trace path imports antenv.axon_hooks, which this
    container's antenv lacks. Install a working ctypes-based NTFF hook (or a
    None hook) so BASS_TRACE=1 degrades gracefully instead of crashing."""
    try:
        import antenv.axon_hooks  # noqa: F401
        return
    except ImportError:
        pass
    import sys as _sys
    import types as _types
    hook = None
    try:
        if "/root/.axon_site" not in _sys.path:
            _sys.path.insert(0, "/root/.axon_site")
        from trn_agent_boot.trn_boot import _ntff_profile_via_ctypes
        hook = _ntff_profile_via_ctypes("/opt/axon/libaxon_pjrt.so")
    except Exception:
        hook = None
    mod = _types.ModuleType("antenv.axon_hooks")
    mod.get_axon_ntff_profile_hook = lambda: hook
    mod.set_axon_ntff_profile_hook = lambda h: None
    _sys.modules["antenv.axon_hooks"] = mod
    try:
        import concourse.bass_utils as _bu
        _bu.upload_artifacts = lambda tmpdir: f"local:{tmpdir}"
    except Exception:
        pass


_ensure_trace_shim()


F32 = mybir.dt.float32
F32R = mybir.dt.float32r
BF16 = mybir.dt.bfloat16
FP16 = mybir.dt.float16

B, S, D, H = 4, 2048, 1024, 16
DH = D // H
N_CORES = 8
PAIRS = 4                  # head pairs per core (8 heads)
DINC = 8                   # 128-wide din chunks
KC = S // 128              # 16 key chunks
QT = S // 512              # 4 query tiles
SEG = 193                  # per-pair vaug segment (65 + 128)
VSEG = PAIRS * SEG         # 772 per key chunk

_ws_counter = 0


def _split_multi_waits(nc):
    """walrus in this env rejects >1 sem wait per instruction; hoist extras
    onto same-engine standalone semaphore-wait instructions."""
    global _ws_counter
    f = nc.m.functions[0]
    for bb in f.blocks:
        insts = bb.instructions  # live list
        i = 0
        while i < len(insts):
            inst = insts[i]
            si = inst.sync_info
            waits = list(si.on_wait) if si is not None and si.on_wait else []
            if len(waits) > 1:
                eng = getattr(inst, "engine", None)
                assert eng is not None and eng in nc.engines, (
                    f"multi-wait on non-engine inst {inst.name} ({type(inst).__name__})"
                )
                for w in waits[:-1]:
                    _ws_counter += 1
                    ev = mybir.InstEventSemaphore(
                        name=f"I-wsplit-{_ws_counter}", ins=[], outs=[]
                    )
                    ev.engine = eng
                    ev.sync_info = mybir.SyncInfo(on_wait=[w], on_update=[])
                    nc.register_instruction(ev, overwrite=True)
                    insts.insert(i, ev)
                    i += 1
                inst.sync_info = mybir.SyncInfo(
                    on_wait=[waits[-1]], on_update=list(si.on_update or [])
                )
            i += 1


def build_bass():
    nc = bass.Bass()
    XT = nc.declare_dram_parameter("XT", [D, S], FP16, isOutput=False)
    WQP = nc.declare_dram_parameter("WQP", [PAIRS, 128, 1024], FP16, isOutput=False)
    WKP = nc.declare_dram_parameter("WKP", [PAIRS, 128, 1024], FP16, isOutput=False)
    WVP = nc.declare_dram_parameter("WVP", [128, 4096], FP16, isOutput=False)
    WOP = nc.declare_dram_parameter("WOP", [PAIRS, 128, 1024], FP16, isOutput=False)
    BQK = nc.declare_dram_parameter("BQK", [128, 2 * PAIRS], F32, isOutput=False)
    BVB = nc.declare_dram_parameter("BVB", [128, 512], F32, isOutput=False)
    Y = nc.declare_dram_parameter("Y", [S, D], BF16, isOutput=True)

    with TileContext(nc) as tc:
        with (
            tc.tile_pool(name="sb", bufs=1) as sb,
            tc.tile_pool(name="ps", bufs=1, space="PSUM") as ps,
        ):
            # ---- constants
            bqk = sb.tile([128, 2 * PAIRS], F32, tag="bqk")
            bvb = sb.tile([128, 512], F32, tag="bvb")
            ones16 = sb.tile([128, 128], FP16, tag="ones16")
            nc.vector.memset(ones16[:, :], 1.0)
            nc.sync.dma_start(out=bqk[:, :], in_=BQK[:, :])
            nc.sync.dma_start(out=bvb[:, :], in_=BVB[:, :])

            # ---- input loads: x in 512-column waves so the V-projection can
            # start early.  The first wave (wv + x cols 0:512) is spread
            # across FOUR issue queues (sync/gpsimd/scalar/vector are all
            # idle at startup) to cut time-to-first-matmul.
            wv_sb = sb.tile([128, 4096], FP16, tag="wv", name="wv_sb")
            nc.sync.dma_start(out=wv_sb[:, 0:2048], in_=WVP[:, 0:2048])
            nc.gpsimd.dma_start(out=wv_sb[:, 2048:4096], in_=WVP[:, 2048:4096])
            xt = [sb.tile([128, S], FP16, tag=f"xt{d}", name=f"xt{d}")
                  for d in range(DINC)]
            w1eng = [nc.sync, nc.gpsimd, nc.scalar, nc.sync,
                     nc.gpsimd, nc.scalar, nc.sync, nc.gpsimd]
            # first key chunk (cols 0:128) alone so V-proj kc=0 starts asap
            for d in range(DINC):
                w1eng[d].dma_start(out=xt[d][:, 0:128], in_=XT[d * 128:(d + 1) * 128, 0:128])
            for d in range(DINC):
                w1eng[d].dma_start(out=xt[d][:, 128:512], in_=XT[d * 128:(d + 1) * 128, 128:512])
            for d in range(DINC):
                w1eng[d].dma_start(out=xt[d][:, 512:1024], in_=XT[d * 128:(d + 1) * 128, 512:1024])
            # pair-0 weights (needed only after the 27us V-proj phase)
            wq_t = [None] * PAIRS
            wk_t = [None] * PAIRS
            wq_t[0] = sb.tile([128, 1024], FP16, tag="wq", bufs=3, name="wq0")
            wk_t[0] = sb.tile([128, 1024], FP16, tag="wk", bufs=3, name="wk0")
            nc.sync.dma_start(out=wq_t[0][:, :], in_=WQP[0, :, :])
            nc.sync.dma_start(out=wk_t[0][:, :], in_=WKP[0, :, :])
            for c0 in range(1024, S, 512):
                for d in range(DINC):
                    w1eng[d].dma_start(out=xt[d][:, c0:c0 + 512],
                                       in_=XT[d * 128:(d + 1) * 128, c0:c0 + 512])
            # output-projection weights, low priority
            wo_sb = sb.tile([128, PAIRS * 1024], FP16, tag="wo", name="wo_sb")
            for j in range(PAIRS):
                nc.gpsimd.dma_start(out=wo_sb[:, j * 1024:(j + 1) * 1024],
                                    in_=WOP[j, :, :])

            # ---- V projection -> augmented V layout, fp16.
            # Per key chunk segment of 772 cols, per pair j at j*193:
            #   [V_h(2j) 64 | ones | ones | junk 63 | V_h(2j+1) 64]
            # psA stationary = cols 0..65 (V_h0|ones): psum row 64 = softmax
            # sums h0.  psB stationary = cols 65..193 (ones|junk|V_h1): psum
            # row 0 = sums h1, rows 64..127 = h1 attention out.
            # Split by pair-half: V01 runs upfront (pair 0 needs it), V23
            # rides the fill queue (needed only from pair 2), so the scalar
            # exp stream starts ~14us earlier.
            vaug = sb.tile([128, KC * VSEG], FP16, tag="vaug", name="vaug")
            vsegs = vaug[:, :].rearrange("p (s c) -> p s c", c=VSEG)
            for j in range(PAIRS):
                nc.vector.memset(vsegs[:, :, j * SEG + 64:j * SEG + 65], 1.0)
                nc.vector.memset(vsegs[:, :, j * SEG + 65:j * SEG + 66], 1.0)

            def gen_vproj(jp, k0=0, k1=KC):
                """V projection for pair-half jp (pairs 2jp, 2jp+1), N=256."""
                for kc in range(k0, k1):
                    vps = ps.tile([128, 256], F32, tag="ps_proj", bufs=2)
                    for d in range(DINC):
                        nc.tensor.matmul(
                            vps[:, :],
                            xt[d][:, kc * 128:(kc + 1) * 128],
                            wv_sb[:, d * 512 + jp * 256:d * 512 + jp * 256 + 256],
                            start=(d == 0), stop=(d == DINC - 1),
                        )
                        if d == 3:
                            yield
                    s0 = kc * VSEG
                    with nc.allow_low_precision(reason="fp16 V"):
                        for jj in range(2):
                            j = 2 * jp + jj
                            o = s0 + j * SEG
                            c = j * 128
                            nc.vector.tensor_add(
                                vaug[:, o:o + 64],
                                vps[:, jj * 128:jj * 128 + 64], bvb[:, c:c + 64])
                            nc.vector.tensor_add(
                                vaug[:, o + 129:o + 193],
                                vps[:, jj * 128 + 64:jj * 128 + 128],
                                bvb[:, c + 64:c + 128])
                    yield

            for _ in gen_vproj(0, 0, 4):
                pass

            qt_pool = [sb.tile([128, S], FP16, tag="qt", bufs=2, name=f"qt{i}") for i in range(2)]
            kt_pool = [sb.tile([128, S], FP16, tag="kt", bufs=2, name=f"kt{i}") for i in range(2)]
            aot = [sb.tile([128, S], FP16, tag=f"ao{j}", name=f"ao{j}")
                   for j in range(PAIRS)]

            def gen_proj(wt, out_t, bias_col, t0=0, t1=QT):
                """Q/K projection tiles [t0,t1) as a generator of small
                PE pieces (2 matmuls each) for interleaving."""
                for tt in range(t0, t1):
                    pp = ps.tile([128, 512], F32, tag="ps_proj", bufs=2)
                    for d0 in range(0, DINC, 2):
                        for d in (d0, d0 + 1):
                            nc.tensor.matmul(
                                pp[:, :],
                                wt[:, d * 128:(d + 1) * 128],
                                xt[d][:, tt * 512:(tt + 1) * 512],
                                start=(d == 0), stop=(d == DINC - 1),
                            )
                        yield
                    with nc.allow_low_precision(reason="fp16 qk"):
                        nc.vector.tensor_scalar_add(
                            out_t[:, tt * 512:(tt + 1) * 512], pp[:, :],
                            bqk[:, bias_col:bias_col + 1],
                        )

            y_tiles = {}

            def gen_oproj(c0, c1):
                """Output-projection token chunks [c0, c1) as PE pieces."""
                for c in range(c0, c1):
                    ysb = sb.tile([128, 1024], BF16, tag="y", bufs=2)
                    y_tiles[c] = ysb
                    for nt in range(2):
                        yps = ps.tile([128, 512], F32, tag="ps_proj", bufs=2)
                        for jj in range(PAIRS):
                            nc.tensor.matmul(
                                yps[:, :],
                                aot[jj][:, c * 128:(c + 1) * 128],
                                wo_sb[:, jj * 1024 + nt * 512: jj * 1024 + nt * 512 + 512],
                                start=(jj == 0), stop=(jj == PAIRS - 1),
                            )
                        with nc.allow_low_precision(reason="bf16 partial out"):
                            nc.vector.tensor_copy(
                                ysb[:, nt * 512:(nt + 1) * 512], yps[:, :])
                        yield
                    nc.gpsimd.dma_start(
                        out=Y[c * 128:(c + 1) * 128, :], in_=ysb[:, :])
                    yield

            # fill machinery: a list of (generator) producers pumped one piece
            # at a time inside the attention loop; closures (tail part B) take
            # priority.
            import collections
            fq = collections.deque()

            def pump():
                while fq:
                    item = fq[0]
                    if callable(item):
                        fq.popleft()
                        item()
                        return
                    try:
                        next(item)
                        return
                    except StopIteration:
                        fq.popleft()
                        continue

            def drain():
                while fq:
                    pump()

            def make_tail(j, qsl, psA, psB):
                """Softmax tail for one (pair, q2).  Part A (inline): stage
                the PV psums to SBUF.  Slot pieces dispatched in the NEXT q2:
                1/den = exp(-ln(den)) on the [1,512] sums rows only (~0.7us
                scalar pieces; ln/exp share one activation table so no table
                reloads), then per head a fp16 ones-matmul broadcast into a
                short-lived ps_proj tile + DVE multiply straight from PSUM.
                """
                # h1's sums are copied into cols 0:512 (same as h0, on
                # partition 0) so the ln/exp span is [65,512], not [65,1024]
                srow = sb.tile([65, 512], F32, tag="srow", bufs=4)
                rrow = sb.tile([65, 512], FP16, tag="rrow", bufs=4)
                aocp = sb.tile([128, 1024], F32, tag="aocp", bufs=4)
                nc.vector.tensor_copy(srow[64:65, 0:512], psA[64:65, :])
                nc.vector.tensor_copy(srow[0:1, 0:512], psB[0:1, :])
                nc.vector.tensor_copy(aocp[0:64, 0:512], psA[0:64, :])
                nc.vector.tensor_copy(aocp[64:128, 512:1024], psB[64:128, :])

                lrow = sb.tile([65, 512], F32, tag="lrow", bufs=4)

                def t_ln():
                    # 1/den = exp(-ln(den)): ln/exp share one activation
                    # table (no reloads); one [65,512] span covers both
                    # heads' sums rows (junk lanes harmlessly processed).
                    # DVE reciprocal is NOT used: at 6.5ns/free-elem the
                    # [1,512] rows cost 3.3us each and saturate the DVE.
                    nc.scalar.activation(lrow[0:65, :], srow[0:65, :],
                                         mybir.ActivationFunctionType.Ln)

                def t_exp():
                    nc.scalar.activation(rrow[0:65, :], lrow[0:65, :],
                                         mybir.ActivationFunctionType.Exp,
                                         scale=-1.0)

                def t_h0():
                    psbc0 = ps.tile([128, 512], F32, tag="ps_proj", bufs=2)
                    nc.tensor.matmul(psbc0[:, :], ones16[64:65, :],
                                     rrow[64:65, 0:512],
                                     start=True, stop=True)
                    with nc.allow_low_precision(reason="fp16 out"):
                        nc.vector.tensor_mul(
                            aot[j][0:64, qsl], aocp[0:64, 0:512], psbc0[0:64, :])

                def t_h1():
                    psbc1 = ps.tile([128, 512], F32, tag="ps_proj", bufs=2)
                    nc.tensor.matmul(psbc1[:, :], ones16[0:1, :],
                                     rrow[0:1, 0:512],
                                     start=True, stop=True)
                    with nc.allow_low_precision(reason="fp16 out"):
                        nc.vector.tensor_mul(
                            aot[j][64:128, qsl], aocp[64:128, 512:1024],
                            psbc1[64:128, :])

                return {0: t_ln, 2: t_exp, 9: t_h0, 11: t_h1}

            # ---- upfront: K0 fully (scores q2=0 need all key tiles) and
            # only the first Q0 tile; Q0 tiles 1-3 and the V23 projection
            # join the fill queue with loose deadlines.
            qt_cur, kt_cur = qt_pool[0], kt_pool[0]
            for _ in gen_proj(wk_t[0], kt_cur, 1):
                pass
            for _ in gen_proj(wq_t[0], qt_cur, 0, 0, 1):
                pass
            # V01 kc>=4 arrives JIT through the double-pumped q2=0 fill
            # (2 pieces/kc keeps it 4 key chunks ahead of the PV consumer);
            # Q0 tiles 1-3 follow before q2=1 needs them.
            fq.append(gen_vproj(0, 4, KC))
            fq.append(gen_proj(wq_t[0], qt_cur, 0, 1, QT))
            fq.append(gen_vproj(1, 0, 8))

            # ---- main loop over head pairs
            tail_pieces = None   # pair 3: pending tail of the previous q2
            pending_tails = []   # pairs 0-2: whole tails for the pair-end drain
            for j in range(PAIRS):
                if j < PAIRS - 1:
                    wq_t[j + 1] = sb.tile([128, 1024], FP16, tag="wq", bufs=3, name=f"wq{j+1}")
                    wk_t[j + 1] = sb.tile([128, 1024], FP16, tag="wk", bufs=3, name=f"wk{j+1}")
                    nc.sync.dma_start(out=wq_t[j + 1][:, :], in_=WQP[j + 1, :, :])
                    nc.sync.dma_start(out=wk_t[j + 1][:, :], in_=WKP[j + 1, :, :])
                    qt_nxt = qt_pool[(j + 1) % 2]
                    kt_nxt = kt_pool[(j + 1) % 2]
                    fq.append(gen_proj(wq_t[j + 1], qt_nxt, 2 * (j + 1)))
                    fq.append(gen_proj(wk_t[j + 1], kt_nxt, 2 * (j + 1) + 1))
                    if j == 0:
                        fq.append(gen_vproj(1, 8, KC))

                for q2 in range(QT):
                    if j == PAIRS - 1 and q2 == 2:
                        # O-proj chunks 0..7 ride along pair 3's q2=2..3;
                        # their aot[3] slices (q2 0..1) are complete by then
                        fq.append(gen_oproj(0, 8))
                    qsl = slice(q2 * 512, (q2 + 1) * 512)
                    psA = ps.tile([65, 512], F32, tag="ps_pv", bufs=2)
                    psB = ps.tile([128, 512], F32, tag="ps_pv", bufs=2)
                    for kc in range(KC):
                        pss = ps.tile([128, 1024], F32, tag="ps_s", bufs=2)
                        ksl = slice(kc * 128, (kc + 1) * 128)
                        nc.tensor.matmul(
                            pss[:, 0:512], kt_cur[0:64, ksl], qt_cur[0:64, qsl],
                            start=True, stop=True,
                        )
                        nc.tensor.matmul(
                            pss[:, 512:1024], kt_cur[64:128, ksl], qt_cur[64:128, qsl],
                            start=True, stop=True,
                        )
                        pt = sb.tile([128, 1024], BF16, tag="pt", bufs=8)
                        nc.scalar.activation(
                            pt[:, :], pss[:, :],
                            mybir.ActivationFunctionType.Exp,
                        )
                        s0 = kc * VSEG + j * SEG
                        nc.tensor.matmul(
                            psA[:, :], vaug[:, s0:s0 + 65], pt[:, 0:512],
                            start=(kc == 0), stop=(kc == KC - 1),
                        )
                        nc.tensor.matmul(
                            psB[:, :], vaug[:, s0 + 65:s0 + 193], pt[:, 512:1024],
                            start=(kc == 0), stop=(kc == KC - 1),
                        )
                        # pair 3 only: previous q2's tail pieces at fixed
                        # slots (timed so the PE bcast never waits on the
                        # scalar ln/exp).  Pairs 0-2 defer whole tails to the
                        # pair-end drain block (their aot is read only in
                        # pair 3), keeping the scalar stream pure exps.
                        # NOTE: FULL matmul fill here raises sustained chip
                        # power and the package DVFS cuts the clock ~20%, a
                        # net loss (measured); the 8-slot fill density stays
                        # under the knee.
                        if tail_pieces:
                            fn = tail_pieces.pop(kc, None)
                            if fn is not None:
                                fn()
                                if not tail_pieces:
                                    tail_pieces = None
                        if j == 0 and q2 == 0:
                            pump()
                            pump()
                        elif kc in (1, 3, 4, 5, 6, 7, 13, 15) or (
                                j <= 1 and kc in (12, 14)) or (
                                j == PAIRS - 1 and kc in (8, 12, 14)):
                            pump()

                    new_tail = make_tail(j, qsl, psA, psB)
                    if j < PAIRS - 1:
                        pending_tails.append(new_tail)
                    elif q2 == QT - 1:
                        final_tail = new_tail
                    else:
                        assert tail_pieces is None
                        tail_pieces = new_tail

                # phased: next pair's projections (and any queued O-proj
                # chunks) run as a block here; the deferred tails' scalar
                # ln/exp runs under this block (the scalar is idle here),
                # then their broadcasts/muls follow the drained PE work.
                for t in pending_tails:
                    t[0]()
                    t[2]()
                drain()
                for t in pending_tails:
                    t[9]()
                    t[11]()
                pending_tails = []
                if j < PAIRS - 1:
                    qt_cur, kt_cur = qt_nxt, kt_nxt

            # ---- endgame: start the final tail's scalar chain at once, then
            # emit O-proj chunks that don't need the last q2 while it runs,
            # then the final broadcasts/muls, then the last chunks.
            final_tail[0]()
            final_tail[2]()
            for _ in gen_oproj(8, 12):
                pass
            final_tail[9]()
            final_tail[11]()
            for _ in gen_oproj(12, 16):
                pass

    _split_multi_waits(nc)
    return nc


_nc_cache = {}
_last_results = None


def _get_nc():
    if "nc" not in _nc_cache:
        _nc_cache["nc"] = build_bass()
    return _nc_cache["nc"]


def _prep_weights(hh, wq, bq, wk, bk, wv, bv, wo):
    """Pack the head-half hh slice (heads hh*8..hh*8+8) of all weights."""
    sl = slice(hh * 512, (hh + 1) * 512)
    scale = np.float32(1.0 / np.sqrt(DH))
    wqT = np.ascontiguousarray(wq.T[:, sl]) * scale   # [1024, 512]
    wkT = np.ascontiguousarray(wk.T[:, sl])
    wvT = np.ascontiguousarray(wv.T[:, sl])
    woT = np.ascontiguousarray(wo.T[sl, :])           # [512, 1024]
    # WQP[j, p, (d m)] = wqT[d*128+p, j*128+m]
    A = wqT.reshape(DINC, 128, PAIRS, 128)
    WQP = np.ascontiguousarray(A.transpose(2, 1, 0, 3).reshape(PAIRS, 128, 1024)).astype(np.float16)
    A = wkT.reshape(DINC, 128, PAIRS, 128)
    WKP = np.ascontiguousarray(A.transpose(2, 1, 0, 3).reshape(PAIRS, 128, 1024)).astype(np.float16)
    # WVP[p, (d n)] = wvT[d*128+p, n]
    A = wvT.reshape(DINC, 128, 512)
    WVP = np.ascontiguousarray(A.transpose(1, 0, 2).reshape(128, 4096)).astype(np.float16)
    # WOP[j, p, n] = woT[j*128+p, n]
    WOP = np.ascontiguousarray(woT.reshape(PAIRS, 128, 1024)).astype(np.float16)
    bqs = (bq[sl] * scale).reshape(PAIRS, 128)
    bkr = bk[sl].reshape(PAIRS, 128)
    BQK = np.empty((128, 2 * PAIRS), np.float32)
    for jx in range(PAIRS):
        BQK[:, 2 * jx] = bqs[jx]
        BQK[:, 2 * jx + 1] = bkr[jx]
    BVB = np.ascontiguousarray(np.tile(bv[sl].reshape(1, 512), (128, 1)))
    return {"WQP": WQP, "WKP": WKP, "WVP": WVP, "WOP": WOP,
            "BQK": BQK, "BVB": BVB}


def kernel(x_input, wq, bq, wk, bk, wv, bv, wo, bo):
    x_input = np.asarray(x_input, dtype=np.float32)
    wq, bq = np.asarray(wq, np.float32), np.asarray(bq, np.float32)
    wk, bk = np.asarray(wk, np.float32), np.asarray(bk, np.float32)
    wv, bv = np.asarray(wv, np.float32), np.asarray(bv, np.float32)
    wo, bo = np.asarray(wo, np.float32), np.asarray(bo, np.float32)

    wsets = [_prep_weights(hh, wq, bq, wk, bk, wv, bv, wo) for hh in range(2)]
    xTs = [np.ascontiguousarray(x_input[b].T).astype(np.float16) for b in range(B)]

    nc = _get_nc()
    in_maps = []
    for c in range(N_CORES):
        m = dict(wsets[c % 2])
        m["XT"] = xTs[c // 2]
        in_maps.append(m)

    res = run_bass_kernel_spmd(nc, in_maps, list(range(N_CORES)))
    global _last_results
    _last_results = res

    out = np.empty((B, S, D), np.float32)
    for b in range(B):
        y0 = np.asarray(res.results[2 * b]["Y"]).astype(np.float32)
        y1 = np.asarray(res.results[2 * b + 1]["Y"]).astype(np.float32)
        out[b] = y0 + y1
    out += bo.reshape(1, 1, D)
    return out


# revision 38
# speedup vs baseline: 1.1671x; 1.1671x over previous
"""Distributed MultiHeadAttention kernel for 8 TRN2 NeuronCores.

Problem: B=4, S=2048, D=1024, H=16, DH=64, fp32 reference, full
(non-causal) attention. ~137 GFLOP total.

Sharding (head-parallel): core c owns batch b=c//2 and head-half hh=c%2
(8 heads = 4 head-pairs, all 2048 queries).  Q/K/V projections are
computed once globally (query-half sharding would duplicate K/V).  Each
core emits a PARTIAL output Y_c = (attn heads_hh) @ wo_hh [2048, 1024]
in bf16; the host sums the two partials per batch and adds the output
bias (the O-projection is linear over head groups), so no cross-core
communication is needed.  One SPMD program; per-core inputs differ only
in data (XT by batch, weight slices by head-half).

Per-core PE stream: 1568 matmuls x 512 moving columns = 803K columns
(~334 us at 2.4 GHz) vs 934K for query-half sharding.  The attention
inner loop is paced by the scalar engine's Exp ([128,1024] PSUM->SBUF
bf16, ~1.11 us per key chunk vs ~0.85 us of PE matmul), so the span is
roughly startup + the saturated exp stream + the endgame.

Schedule (measured on silicon, ~409.6 us vs 480 us for the v1
query-half-sharded kernel; abs-max rel err 2.9e-3):
- Upfront: V-projection for pairs 0-1 only (N=256 per-half split), Q0/K0.
  x arrives in column waves on 3 DMA issue queues; first matmul ~17 us.
- A fill queue interleaves deferred PE work (V23 projection, next pair's
  Q/K projections, first 8 output-projection token chunks) into the
  attention loop at 8 fixed kc slots per 512-query tile -- ~60% of the
  PE idle under the exp pacing.  CRITICAL: filling ALL the idle (or >8
  slots) raises sustained chip power enough that the package DVFS cuts
  the clock ~20% across every engine (exp 1.11->1.33 us), a net LOSS.
  The 8-slot density stays under the knee; leftovers drain as phased
  blocks at pair boundaries where the scalar is idle anyway.
- Softmax: scores for a head pair land in one [128,1024] psum (h0
  contracts on partitions 0-63, h1 on 64-127), ONE Exp per key chunk,
  no max-subtraction (|s| <= ~9, exp <= 6.5e3 fits fp16 P).  PV uses
  augmented stationary tiles ([V_h0|ones] and [ones|junk|V_h1]) so the
  softmax denominators accumulate in psum rows 64 / 0 for free.
- Tail per (pair, q2): 1/den = exp(-ln(den)) on a [65,512] sums-row
  span (both heads' rows copied to cols 0:512; ln/exp share one
  activation table; DVE reciprocal measured 6.5 ns/elem = 3.3 us per
  row and is NOT used), then a K=1 fp16 ones-matmul broadcasts the
  reciprocals and a DVE multiply (reading psum directly) writes aot
  fp16.  Pairs 0-2 defer ENTIRE tails to their pair-end drain blocks
  (aot is only read in pair 3), keeping the scalar stream pure exps;
  pair 3's tails are slot-dispatched into the next q2 so the in-order
  PE queue never waits on them.
- Output projection: per 128-token chunk, 4 accumulating matmuls into
  psum, bf16 copy, DMA out; chunks 0-7 ride pair 3's fill slots, 8-11
  cover the final tail's latency, 12-15 follow it.
- walrus in this environment rejects >1 semaphore wait per instruction;
  a post-pass hoists extra waits onto standalone InstEventSemaphore.
"""
import numpy as np
import ml_dtypes
import concourse.bass as bass
import concourse.mybir as mybir
from concourse.tile import TileContext
from concourse.bass_utils import run_bass_kernel_spmd


def _ensure_trace_shim():
    """concourse's axon trace path imports antenv.axon_hooks, which this
    container's antenv lacks. Install a working ctypes-based NTFF hook (or a
    None hook) so BASS_TRACE=1 degrades gracefully instead of crashing."""
    try:
        import antenv.axon_hooks  # noqa: F401
        return
    except ImportError:
        pass
    import sys as _sys
    import types as _types
    hook = None
    try:
        if "/root/.axon_site" not in _sys.path:
            _sys.path.insert(0, "/root/.axon_site")
        from trn_agent_boot.trn_boot import _ntff_profile_via_ctypes
        hook = _ntff_profile_via_ctypes("/opt/axon/libaxon_pjrt.so")
    except Exception:
        hook = None
    mod = _types.ModuleType("antenv.axon_hooks")
    mod.get_axon_ntff_profile_hook = lambda: hook
    mod.set_axon_ntff_profile_hook = lambda h: None
    _sys.modules["antenv.axon_hooks"] = mod
    try:
        import concourse.bass_utils as _bu
        _bu.upload_artifacts = lambda tmpdir: f"local:{tmpdir}"
    except Exception:
        pass


_ensure_trace_shim()


F32 = mybir.dt.float32
F32R = mybir.dt.float32r
BF16 = mybir.dt.bfloat16
FP16 = mybir.dt.float16

B, S, D, H = 4, 2048, 1024, 16
DH = D // H
N_CORES = 8
PAIRS = 4                  # head pairs per core (8 heads)
DINC = 8                   # 128-wide din chunks
KC = S // 128              # 16 key chunks
QT = S // 512              # 4 query tiles
SEG = 193                  # per-pair vaug segment (65 + 128)
VSEG = PAIRS * SEG         # 772 per key chunk

_ws_counter = 0


def _split_multi_waits(nc):
    """walrus in this env rejects >1 sem wait per instruction; hoist extras
    onto same-engine standalone semaphore-wait instructions."""
    global _ws_counter
    f = nc.m.functions[0]
    for bb in f.blocks:
        insts = bb.instructions  # live list
        i = 0
        while i < len(insts):
            inst = insts[i]
            si = inst.sync_info
            waits = list(si.on_wait) if si is not None and si.on_wait else []
            if len(waits) > 1:
                eng = getattr(inst, "engine", None)
                assert eng is not None and eng in nc.engines, (
                    f"multi-wait on non-engine inst {inst.name} ({type(inst).__name__})"
                )
                for w in waits[:-1]:
                    _ws_counter += 1
                    ev = mybir.InstEventSemaphore(
                        name=f"I-wsplit-{_ws_counter}", ins=[], outs=[]
                    )
                    ev.engine = eng
                    ev.sync_info = mybir.SyncInfo(on_wait=[w], on_update=[])
                    nc.register_instruction(ev, overwrite=True)
                    insts.insert(i, ev)
                    i += 1
                inst.sync_info = mybir.SyncInfo(
                    on_wait=[waits[-1]], on_update=list(si.on_update or [])
                )
            i += 1


def build_bass():
    nc = bass.Bass()
    XT = nc.declare_dram_parameter("XT", [D, S], FP16, isOutput=False)
    WQP = nc.declare_dram_parameter("WQP", [PAIRS, 128, 1024], FP16, isOutput=False)
    WKP = nc.declare_dram_parameter("WKP", [PAIRS, 128, 1024], FP16, isOutput=False)
    WVP = nc.declare_dram_parameter("WVP", [128, 4096], FP16, isOutput=False)
    WOP = nc.declare_dram_parameter("WOP", [PAIRS, 128, 1024], FP16, isOutput=False)
    BQK = nc.declare_dram_parameter("BQK", [128, 2 * PAIRS], F32, isOutput=False)
    BVB = nc.declare_dram_parameter("BVB", [128, 512], F32, isOutput=False)
    Y = nc.declare_dram_parameter("Y", [S, D], BF16, isOutput=True)

    with TileContext(nc) as tc:
        with (
            tc.tile_pool(name="sb", bufs=1) as sb,
            tc.tile_pool(name="ps", bufs=1, space="PSUM") as ps,
        ):
            # ---- constants
            bqk = sb.tile([128, 2 * PAIRS], F32, tag="bqk")
            bvb = sb.tile([128, 512], F32, tag="bvb")
            ones16 = sb.tile([128, 128], FP16, tag="ones16")
            nc.vector.memset(ones16[:, :], 1.0)
            nc.sync.dma_start(out=bqk[:, :], in_=BQK[:, :])
            nc.sync.dma_start(out=bvb[:, :], in_=BVB[:, :])

            # ---- input loads: x in 512-column waves so the V-projection can
            # start early.  The first wave (wv + x cols 0:512) is spread
            # across FOUR issue queues (sync/gpsimd/scalar/vector are all
            # idle at startup) to cut time-to-first-matmul.
            wv_sb = sb.tile([128, 4096], FP16, tag="wv", name="wv_sb")
            nc.sync.dma_start(out=wv_sb[:, 0:2048], in_=WVP[:, 0:2048])
            nc.gpsimd.dma_start(out=wv_sb[:, 2048:4096], in_=WVP[:, 2048:4096])
            xt = [sb.tile([128, S], FP16, tag=f"xt{d}", name=f"xt{d}")
                  for d in range(DINC)]
            w1eng = [nc.sync, nc.gpsimd, nc.scalar, nc.sync,
                     nc.gpsimd, nc.scalar, nc.sync, nc.gpsimd]
            # first key chunk (cols 0:128) alone so V-proj kc=0 starts asap
            for d in range(DINC):
                w1eng[d].dma_start(out=xt[d][:, 0:128], in_=XT[d * 128:(d + 1) * 128, 0:128])
            for d in range(DINC):
                w1eng[d].dma_start(out=xt[d][:, 128:512], in_=XT[d * 128:(d + 1) * 128, 128:512])
            for d in range(DINC):
                w1eng[d].dma_start(out=xt[d][:, 512:1024], in_=XT[d * 128:(d + 1) * 128, 512:1024])
            # pair-0 weights (needed only after the 27us V-proj phase)
            wq_t = [None] * PAIRS
            wk_t = [None] * PAIRS
            wq_t[0] = sb.tile([128, 1024], FP16, tag="wq", bufs=3, name="wq0")
            wk_t[0] = sb.tile([128, 1024], FP16, tag="wk", bufs=3, name="wk0")
            nc.sync.dma_start(out=wq_t[0][:, :], in_=WQP[0, :, :])
            nc.sync.dma_start(out=wk_t[0][:, :], in_=WKP[0, :, :])
            for c0 in range(1024, S, 512):
                for d in range(DINC):
                    w1eng[d].dma_start(out=xt[d][:, c0:c0 + 512],
                                       in_=XT[d * 128:(d + 1) * 128, c0:c0 + 512])
            # output-projection weights, low priority
            wo_sb = sb.tile([128, PAIRS * 1024], FP16, tag="wo", name="wo_sb")
            for j in range(PAIRS):
                nc.gpsimd.dma_start(out=wo_sb[:, j * 1024:(j + 1) * 1024],
                                    in_=WOP[j, :, :])

            # ---- V projection -> augmented V layout, fp16.
            # Per key chunk segment of 772 cols, per pair j at j*193:
            #   [V_h(2j) 64 | ones | ones | junk 63 | V_h(2j+1) 64]
            # psA stationary = cols 0..65 (V_h0|ones): psum row 64 = softmax
            # sums h0.  psB stationary = cols 65..193 (ones|junk|V_h1): psum
            # row 0 = sums h1, rows 64..127 = h1 attention out.
            # Split by pair-half: V01 runs upfront (pair 0 needs it), V23
            # rides the fill queue (needed only from pair 2), so the scalar
            # exp stream starts ~14us earlier.
            vaug = sb.tile([128, KC * VSEG], FP16, tag="vaug", name="vaug")
            vsegs = vaug[:, :].rearrange("p (s c) -> p s c", c=VSEG)
            for j in range(PAIRS):
                nc.vector.memset(vsegs[:, :, j * SEG + 64:j * SEG + 65], 1.0)
                nc.vector.memset(vsegs[:, :, j * SEG + 65:j * SEG + 66], 1.0)

            def gen_vproj(jp, k0=0, k1=KC):
                """V projection for pair-half jp (pairs 2jp, 2jp+1), N=256."""
                for kc in range(k0, k1):
                    vps = ps.tile([128, 256], F32, tag="ps_proj", bufs=2)
                    for d in range(DINC):
                        nc.tensor.matmul(
                            vps[:, :],
                            xt[d][:, kc * 128:(kc + 1) * 128],
                            wv_sb[:, d * 512 + jp * 256:d * 512 + jp * 256 + 256],
                            start=(d == 0), stop=(d == DINC - 1),
                        )
                        if d == 3:
                            yield
                    s0 = kc * VSEG
                    with nc.allow_low_precision(reason="fp16 V"):
                        for jj in range(2):
                            j = 2 * jp + jj
                            o = s0 + j * SEG
                            c = j * 128
                            nc.vector.tensor_add(
                                vaug[:, o:o + 64],
                                vps[:, jj * 128:jj * 128 + 64], bvb[:, c:c + 64])
                            nc.vector.tensor_add(
                                vaug[:, o + 129:o + 193],
                                vps[:, jj * 128 + 64:jj * 128 + 128],
                                bvb[:, c + 64:c + 128])
                    yield

            for _ in gen_vproj(0):
                pass

            qt_pool = [sb.tile([128, S], FP16, tag="qt", bufs=2, name=f"qt{i}") for i in range(2)]
            kt_pool = [sb.tile([128, S], FP16, tag="kt", bufs=2, name=f"kt{i}") for i in range(2)]
            aot = [sb.tile([128, S], FP16, tag=f"ao{j}", name=f"ao{j}")
                   for j in range(PAIRS)]

            def gen_proj(wt, out_t, bias_col, t0=0, t1=QT):
                """Q/K projection tiles [t0,t1) as a generator of small
                PE pieces (2 matmuls each) for interleaving."""
                for tt in range(t0, t1):
                    pp = ps.tile([128, 512], F32, tag="ps_proj", bufs=2)
                    for d0 in range(0, DINC, 2):
                        for d in (d0, d0 + 1):
                            nc.tensor.matmul(
                                pp[:, :],
                                wt[:, d * 128:(d + 1) * 128],
                                xt[d][:, tt * 512:(tt + 1) * 512],
                                start=(d == 0), stop=(d == DINC - 1),
                            )
                        yield
                    with nc.allow_low_precision(reason="fp16 qk"):
                        nc.vector.tensor_scalar_add(
                            out_t[:, tt * 512:(tt + 1) * 512], pp[:, :],
                            bqk[:, bias_col:bias_col + 1],
                        )

            y_tiles = {}

            def gen_oproj(c0, c1):
                """Output-projection token chunks [c0, c1) as PE pieces."""
                for c in range(c0, c1):
                    ysb = sb.tile([128, 1024], BF16, tag="y", bufs=2)
                    y_tiles[c] = ysb
                    for nt in range(2):
                        yps = ps.tile([128, 512], F32, tag="ps_proj", bufs=2)
                        for jj in range(PAIRS):
                            nc.tensor.matmul(
                                yps[:, :],
                                aot[jj][:, c * 128:(c + 1) * 128],
                                wo_sb[:, jj * 1024 + nt * 512: jj * 1024 + nt * 512 + 512],
                                start=(jj == 0), stop=(jj == PAIRS - 1),
                            )
                        with nc.allow_low_precision(reason="bf16 partial out"):
                            nc.vector.tensor_copy(
                                ysb[:, nt * 512:(nt + 1) * 512], yps[:, :])
                        yield
                    nc.gpsimd.dma_start(
                        out=Y[c * 128:(c + 1) * 128, :], in_=ysb[:, :])
                    yield

            # fill machinery: a list of (generator) producers pumped one piece
            # at a time inside the attention loop; closures (tail part B) take
            # priority.
            import collections
            fq = collections.deque()

            def pump():
                while fq:
                    item = fq[0]
                    if callable(item):
                        fq.popleft()
                        item()
                        return
                    try:
                        next(item)
                        return
                    except StopIteration:
                        fq.popleft()
                        continue

            def drain():
                while fq:
                    pump()

            def make_tail(j, qsl, psA, psB):
                """Softmax tail for one (pair, q2).  Part A (inline): stage
                the PV psums to SBUF.  Slot pieces dispatched in the NEXT q2:
                1/den = exp(-ln(den)) on the [1,512] sums rows only (~0.7us
                scalar pieces; ln/exp share one activation table so no table
                reloads), then per head a fp16 ones-matmul broadcast into a
                short-lived ps_proj tile + DVE multiply straight from PSUM.
                """
                # h1's sums are copied into cols 0:512 (same as h0, on
                # partition 0) so the ln/exp span is [65,512], not [65,1024]
                srow = sb.tile([65, 512], F32, tag="srow", bufs=4)
                rrow = sb.tile([65, 512], FP16, tag="rrow", bufs=4)
                aocp = sb.tile([128, 1024], F32, tag="aocp", bufs=4)
                nc.vector.tensor_copy(srow[64:65, 0:512], psA[64:65, :])
                nc.vector.tensor_copy(srow[0:1, 0:512], psB[0:1, :])
                nc.vector.tensor_copy(aocp[0:64, 0:512], psA[0:64, :])
                nc.vector.tensor_copy(aocp[64:128, 512:1024], psB[64:128, :])

                lrow = sb.tile([65, 512], F32, tag="lrow", bufs=4)

                def t_ln():
                    # 1/den = exp(-ln(den)): ln/exp share one activation
                    # table (no reloads); one [65,512] span covers both
                    # heads' sums rows (junk lanes harmlessly processed).
                    # DVE reciprocal is NOT used: at 6.5ns/free-elem the
                    # [1,512] rows cost 3.3us each and saturate the DVE.
                    nc.scalar.activation(lrow[0:65, :], srow[0:65, :],
                                         mybir.ActivationFunctionType.Ln)

                def t_exp():
                    nc.scalar.activation(rrow[0:65, :], lrow[0:65, :],
                                         mybir.ActivationFunctionType.Exp,
                                         scale=-1.0)

                def t_h0():
                    psbc0 = ps.tile([128, 512], F32, tag="ps_proj", bufs=2)
                    nc.tensor.matmul(psbc0[:, :], ones16[64:65, :],
                                     rrow[64:65, 0:512],
                                     start=True, stop=True)
                    with nc.allow_low_precision(reason="fp16 out"):
                        nc.vector.tensor_mul(
                            aot[j][0:64, qsl], aocp[0:64, 0:512], psbc0[0:64, :])

                def t_h1():
                    psbc1 = ps.tile([128, 512], F32, tag="ps_proj", bufs=2)
                    nc.tensor.matmul(psbc1[:, :], ones16[0:1, :],
                                     rrow[0:1, 0:512],
                                     start=True, stop=True)
                    with nc.allow_low_precision(reason="fp16 out"):
                        nc.vector.tensor_mul(
                            aot[j][64:128, qsl], aocp[64:128, 512:1024],
                            psbc1[64:128, :])

                return {0: t_ln, 2: t_exp, 9: t_h0, 11: t_h1}

            # ---- upfront: K0 fully (scores q2=0 need all key tiles) and
            # only the first Q0 tile; Q0 tiles 1-3 and the V23 projection
            # join the fill queue with loose deadlines.
            qt_cur, kt_cur = qt_pool[0], kt_pool[0]
            for _ in gen_proj(wk_t[0], kt_cur, 1, 0, 3):
                pass
            for _ in gen_proj(wq_t[0], qt_cur, 0, 0, 3):
                pass
            # last K0/Q0 tiles via high-priority fill: K0 t3 is needed by
            # kc=12 of q2=0, delivered by the slots around iter 6
            fq.append(gen_proj(wk_t[0], kt_cur, 1, 3, QT))
            fq.append(gen_proj(wq_t[0], qt_cur, 0, 3, QT))
            fq.append(gen_vproj(1, 0, 8))

            # ---- main loop over head pairs
            tail_pieces = None   # pair 3: pending tail of the previous q2
            pending_tails = []   # pairs 0-2: whole tails for the pair-end drain
            for j in range(PAIRS):
                if j < PAIRS - 1:
                    wq_t[j + 1] = sb.tile([128, 1024], FP16, tag="wq", bufs=3, name=f"wq{j+1}")
                    wk_t[j + 1] = sb.tile([128, 1024], FP16, tag="wk", bufs=3, name=f"wk{j+1}")
                    nc.sync.dma_start(out=wq_t[j + 1][:, :], in_=WQP[j + 1, :, :])
                    nc.sync.dma_start(out=wk_t[j + 1][:, :], in_=WKP[j + 1, :, :])
                    qt_nxt = qt_pool[(j + 1) % 2]
                    kt_nxt = kt_pool[(j + 1) % 2]
                    fq.append(gen_proj(wq_t[j + 1], qt_nxt, 2 * (j + 1)))
                    fq.append(gen_proj(wk_t[j + 1], kt_nxt, 2 * (j + 1) + 1))
                    if j == 0:
                        fq.append(gen_vproj(1, 8, KC))

                for q2 in range(QT):
                    if j == PAIRS - 1 and q2 == 2:
                        # O-proj chunks 0..7 ride along pair 3's q2=2..3;
                        # their aot[3] slices (q2 0..1) are complete by then
                        fq.append(gen_oproj(0, 8))
                    qsl = slice(q2 * 512, (q2 + 1) * 512)
                    psA = ps.tile([65, 512], F32, tag="ps_pv", bufs=2)
                    psB = ps.tile([128, 512], F32, tag="ps_pv", bufs=2)
                    for kc in range(KC):
                        pss = ps.tile([128, 1024], F32, tag="ps_s", bufs=2)
                        ksl = slice(kc * 128, (kc + 1) * 128)
                        nc.tensor.matmul(
                            pss[:, 0:512], kt_cur[0:64, ksl], qt_cur[0:64, qsl],
                            start=True, stop=True,
                        )
                        nc.tensor.matmul(
                            pss[:, 512:1024], kt_cur[64:128, ksl], qt_cur[64:128, qsl],
                            start=True, stop=True,
                        )
                        pt = sb.tile([128, 1024], BF16, tag="pt", bufs=8)
                        nc.scalar.activation(
                            pt[:, :], pss[:, :],
                            mybir.ActivationFunctionType.Exp,
                        )
                        s0 = kc * VSEG + j * SEG
                        nc.tensor.matmul(
                            psA[:, :], vaug[:, s0:s0 + 65], pt[:, 0:512],
                            start=(kc == 0), stop=(kc == KC - 1),
                        )
                        nc.tensor.matmul(
                            psB[:, :], vaug[:, s0 + 65:s0 + 193], pt[:, 512:1024],
                            start=(kc == 0), stop=(kc == KC - 1),
                        )
                        # pair 3 only: previous q2's tail pieces at fixed
                        # slots (timed so the PE bcast never waits on the
                        # scalar ln/exp).  Pairs 0-2 defer whole tails to the
                        # pair-end drain block (their aot is read only in
                        # pair 3), keeping the scalar stream pure exps.
                        # NOTE: FULL matmul fill here raises sustained chip
                        # power and the package DVFS cuts the clock ~20%, a
                        # net loss (measured); the 8-slot fill density stays
                        # under the knee.
                        if tail_pieces:
                            fn = tail_pieces.pop(kc, None)
                            if fn is not None:
                                fn()
                                if not tail_pieces:
                                    tail_pieces = None
                        if kc in (1, 3, 4, 5, 6, 7, 13, 15) or (
                                j <= 1 and kc in (12, 14)) or (
                                j == PAIRS - 1 and kc in (8, 12, 14)):
                            pump()

                    new_tail = make_tail(j, qsl, psA, psB)
                    if j < PAIRS - 1:
                        pending_tails.append(new_tail)
                    elif q2 == QT - 1:
                        final_tail = new_tail
                    else:
                        assert tail_pieces is None
                        tail_pieces = new_tail

                # phased: next pair's projections (and any queued O-proj
                # chunks) run as a block here; the deferred tails' scalar
                # ln/exp runs under this block (the scalar is idle here),
                # then their broadcasts/muls follow the drained PE work.
                for t in pending_tails:
                    t[0]()
                    t[2]()
                drain()
                for t in pending_tails:
                    t[9]()
                    t[11]()
                pending_tails = []
                if j < PAIRS - 1:
                    qt_cur, kt_cur = qt_nxt, kt_nxt

            # ---- endgame: start the final tail's scalar chain at once, then
            # emit O-proj chunks that don't need the last q2 while it runs,
            # then the final broadcasts/muls, then the last chunks.
            final_tail[0]()
            final_tail[2]()
            for _ in gen_oproj(8, 12):
                pass
            final_tail[9]()
            final_tail[11]()
            for _ in gen_oproj(12, 16):
                pass

    _split_multi_waits(nc)
    return nc


_nc_cache = {}
_last_results = None


def _get_nc():
    if "nc" not in _nc_cache:
        _nc_cache["nc"] = build_bass()
    return _nc_cache["nc"]


def _prep_weights(hh, wq, bq, wk, bk, wv, bv, wo):
    """Pack the head-half hh slice (heads hh*8..hh*8+8) of all weights."""
    sl = slice(hh * 512, (hh + 1) * 512)
    scale = np.float32(1.0 / np.sqrt(DH))
    wqT = np.ascontiguousarray(wq.T[:, sl]) * scale   # [1024, 512]
    wkT = np.ascontiguousarray(wk.T[:, sl])
    wvT = np.ascontiguousarray(wv.T[:, sl])
    woT = np.ascontiguousarray(wo.T[sl, :])           # [512, 1024]
    # WQP[j, p, (d m)] = wqT[d*128+p, j*128+m]
    A = wqT.reshape(DINC, 128, PAIRS, 128)
    WQP = np.ascontiguousarray(A.transpose(2, 1, 0, 3).reshape(PAIRS, 128, 1024)).astype(np.float16)
    A = wkT.reshape(DINC, 128, PAIRS, 128)
    WKP = np.ascontiguousarray(A.transpose(2, 1, 0, 3).reshape(PAIRS, 128, 1024)).astype(np.float16)
    # WVP[p, (d n)] = wvT[d*128+p, n]
    A = wvT.reshape(DINC, 128, 512)
    WVP = np.ascontiguousarray(A.transpose(1, 0, 2).reshape(128, 4096)).astype(np.float16)
    # WOP[j, p, n] = woT[j*128+p, n]
    WOP = np.ascontiguousarray(woT.reshape(PAIRS, 128, 1024)).astype(np.float16)
    bqs = (bq[sl] * scale).reshape(PAIRS, 128)
    bkr = bk[sl].reshape(PAIRS, 128)
    BQK = np.empty((128, 2 * PAIRS), np.float32)
    for jx in range(PAIRS):
        BQK[:, 2 * jx] = bqs[jx]
        BQK[:, 2 * jx + 1] = bkr[jx]
    BVB = np.ascontiguousarray(np.tile(bv[sl].reshape(1, 512), (128, 1)))
    return {"WQP": WQP, "WKP": WKP, "WVP": WVP, "WOP": WOP,
            "BQK": BQK, "BVB": BVB}


def kernel(x_input, wq, bq, wk, bk, wv, bv, wo, bo):
    x_input = np.asarray(x_input, dtype=np.float32)
    wq, bq = np.asarray(wq, np.float32), np.asarray(bq, np.float32)
    wk, bk = np.asarray(wk, np.float32), np.asarray(bk, np.float32)
    wv, bv = np.asarray(wv, np.float32), np.asarray(bv, np.float32)
    wo, bo = np.asarray(wo, np.float32), np.asarray(bo, np.float32)

    wsets = [_prep_weights(hh, wq, bq, wk, bk, wv, bv, wo) for hh in range(2)]
    xTs = [np.ascontiguousarray(x_input[b].T).astype(np.float16) for b in range(B)]

    nc = _get_nc()
    in_maps = []
    for c in range(N_CORES):
        m = dict(wsets[c % 2])
        m["XT"] = xTs[c // 2]
        in_maps.append(m)

    res = run_bass_kernel_spmd(nc, in_maps, list(range(N_CORES)))
    global _last_results
    _last_results = res

    out = np.empty((B, S, D), np.float32)
    for b in range(B):
        y0 = np.asarray(res.results[2 * b]["Y"]).astype(np.float32)
        y1 = np.asarray(res.results[2 * b + 1]["Y"]).astype(np.float32)
        out[b] = y0 + y1
    out += bo.reshape(1, 1, D)
    return out


# revision 39
# speedup vs baseline: 1.1764x; 1.0079x over previous
"""Distributed MultiHeadAttention kernel for 8 TRN2 NeuronCores.

Problem: B=4, S=2048, D=1024, H=16, DH=64, fp32 reference, full
(non-causal) attention. ~137 GFLOP total.

Sharding (head-parallel): core c owns batch b=c//2 and head-half hh=c%2
(8 heads = 4 head-pairs, all 2048 queries).  Q/K/V projections are
computed once globally (query-half sharding would duplicate K/V).  Each
core emits a PARTIAL output Y_c = (attn heads_hh) @ wo_hh [2048, 1024]
in bf16; the host sums the two partials per batch and adds the output
bias (the O-projection is linear over head groups), so no cross-core
communication is needed.  One SPMD program; per-core inputs differ only
in data (XT by batch, weight slices by head-half).

Per-core PE stream: 1568 matmuls x 512 moving columns = 803K columns
(~334 us at 2.4 GHz) vs 934K for query-half sharding.  The attention
inner loop is paced by the scalar engine's Exp ([128,1024] PSUM->SBUF
bf16, ~1.11 us per key chunk vs ~0.85 us of PE matmul), so the span is
roughly startup + the saturated exp stream + the endgame.

Schedule (measured on silicon, ~409.6 us vs 480 us for the v1
query-half-sharded kernel; abs-max rel err 2.9e-3):
- Upfront: V-projection for pairs 0-1 only (N=256 per-half split), Q0/K0.
  x arrives in column waves on 3 DMA issue queues; first matmul ~17 us.
- A fill queue interleaves deferred PE work (V23 projection, next pair's
  Q/K projections, first 8 output-projection token chunks) into the
  attention loop at 8 fixed kc slots per 512-query tile -- ~60% of the
  PE idle under the exp pacing.  CRITICAL: filling ALL the idle (or >8
  slots) raises sustained chip power enough that the package DVFS cuts
  the clock ~20% across every engine (exp 1.11->1.33 us), a net LOSS.
  The 8-slot density stays under the knee; leftovers drain as phased
  blocks at pair boundaries where the scalar is idle anyway.
- Softmax: scores for a head pair land in one [128,1024] psum (h0
  contracts on partitions 0-63, h1 on 64-127), ONE Exp per key chunk,
  no max-subtraction (|s| <= ~9, exp <= 6.5e3 fits fp16 P).  PV uses
  augmented stationary tiles ([V_h0|ones] and [ones|junk|V_h1]) so the
  softmax denominators accumulate in psum rows 64 / 0 for free.
- Tail per (pair, q2): 1/den = exp(-ln(den)) on a [65,512] sums-row
  span (both heads' rows copied to cols 0:512; ln/exp share one
  activation table; DVE reciprocal measured 6.5 ns/elem = 3.3 us per
  row and is NOT used), then a K=1 fp16 ones-matmul broadcasts the
  reciprocals and a DVE multiply (reading psum directly) writes aot
  fp16.  Pairs 0-2 defer ENTIRE tails to their pair-end drain blocks
  (aot is only read in pair 3), keeping the scalar stream pure exps;
  pair 3's tails are slot-dispatched into the next q2 so the in-order
  PE queue never waits on them.
- Output projection: per 128-token chunk, 4 accumulating matmuls into
  psum, bf16 copy, DMA out; chunks 0-7 ride pair 3's fill slots, 8-11
  cover the final tail's latency, 12-15 follow it.
- walrus in this environment rejects >1 semaphore wait per instruction;
  a post-pass hoists extra waits onto standalone InstEventSemaphore.
"""
import numpy as np
import ml_dtypes
import concourse.bass as bass
import concourse.mybir as mybir
from concourse.tile import TileContext
from concourse.bass_utils import run_bass_kernel_spmd


def _ensure_trace_shim():
    """concourse's axon trace path imports antenv.axon_hooks, which this
    container's antenv lacks. Install a working ctypes-based NTFF hook (or a
    None hook) so BASS_TRACE=1 degrades gracefully instead of crashing."""
    try:
        import antenv.axon_hooks  # noqa: F401
        return
    except ImportError:
        pass
    import sys as _sys
    import types as _types
    hook = None
    try:
        if "/root/.axon_site" not in _sys.path:
            _sys.path.insert(0, "/root/.axon_site")
        from trn_agent_boot.trn_boot import _ntff_profile_via_ctypes
        hook = _ntff_profile_via_ctypes("/opt/axon/libaxon_pjrt.so")
    except Exception:
        hook = None
    mod = _types.ModuleType("antenv.axon_hooks")
    mod.get_axon_ntff_profile_hook = lambda: hook
    mod.set_axon_ntff_profile_hook = lambda h: None
    _sys.modules["antenv.axon_hooks"] = mod
    try:
        import concourse.bass_utils as _bu
        _bu.upload_artifacts = lambda tmpdir: f"local:{tmpdir}"
    except Exception:
        pass


_ensure_trace_shim()


F32 = mybir.dt.float32
F32R = mybir.dt.float32r
BF16 = mybir.dt.bfloat16
FP16 = mybir.dt.float16

B, S, D, H = 4, 2048, 1024, 16
DH = D // H
N_CORES = 8
PAIRS = 4                  # head pairs per core (8 heads)
DINC = 8                   # 128-wide din chunks
KC = S // 128              # 16 key chunks
QT = S // 512              # 4 query tiles
SEG = 193                  # per-pair vaug segment (65 + 128)
VSEG = PAIRS * SEG         # 772 per key chunk

_ws_counter = 0


def _split_multi_waits(nc):
    """walrus in this env rejects >1 sem wait per instruction; hoist extras
    onto same-engine standalone semaphore-wait instructions."""
    global _ws_counter
    f = nc.m.functions[0]
    for bb in f.blocks:
        insts = bb.instructions  # live list
        i = 0
        while i < len(insts):
            inst = insts[i]
            si = inst.sync_info
            waits = list(si.on_wait) if si is not None and si.on_wait else []
            if len(waits) > 1:
                eng = getattr(inst, "engine", None)
                assert eng is not None and eng in nc.engines, (
                    f"multi-wait on non-engine inst {inst.name} ({type(inst).__name__})"
                )
                for w in waits[:-1]:
                    _ws_counter += 1
                    ev = mybir.InstEventSemaphore(
                        name=f"I-wsplit-{_ws_counter}", ins=[], outs=[]
                    )
                    ev.engine = eng
                    ev.sync_info = mybir.SyncInfo(on_wait=[w], on_update=[])
                    nc.register_instruction(ev, overwrite=True)
                    insts.insert(i, ev)
                    i += 1
                inst.sync_info = mybir.SyncInfo(
                    on_wait=[waits[-1]], on_update=list(si.on_update or [])
                )
            i += 1


def build_bass():
    nc = bass.Bass()
    XT = nc.declare_dram_parameter("XT", [D, S], FP16, isOutput=False)
    WQP = nc.declare_dram_parameter("WQP", [PAIRS, 128, 1024], FP16, isOutput=False)
    WKP = nc.declare_dram_parameter("WKP", [PAIRS, 128, 1024], FP16, isOutput=False)
    WVP = nc.declare_dram_parameter("WVP", [128, 4096], FP16, isOutput=False)
    WOP = nc.declare_dram_parameter("WOP", [PAIRS, 128, 1024], FP16, isOutput=False)
    BQK = nc.declare_dram_parameter("BQK", [128, 2 * PAIRS], F32, isOutput=False)
    BVB = nc.declare_dram_parameter("BVB", [128, 512], F32, isOutput=False)
    Y = nc.declare_dram_parameter("Y", [S, D], BF16, isOutput=True)

    with TileContext(nc) as tc:
        with (
            tc.tile_pool(name="sb", bufs=1) as sb,
            tc.tile_pool(name="ps", bufs=1, space="PSUM") as ps,
        ):
            # ---- constants
            bqk = sb.tile([128, 2 * PAIRS], F32, tag="bqk")
            bvb = sb.tile([128, 512], F32, tag="bvb")
            ones16 = sb.tile([128, 128], FP16, tag="ones16")
            nc.vector.memset(ones16[:, :], 1.0)
            nc.sync.dma_start(out=bqk[:, :], in_=BQK[:, :])
            nc.sync.dma_start(out=bvb[:, :], in_=BVB[:, :])

            # ---- input loads: x in 512-column waves so the V-projection can
            # start early.  The first wave (wv + x cols 0:512) is spread
            # across FOUR issue queues (sync/gpsimd/scalar/vector are all
            # idle at startup) to cut time-to-first-matmul.
            wv_sb = sb.tile([128, 4096], FP16, tag="wv", name="wv_sb")
            nc.sync.dma_start(out=wv_sb[:, 0:2048], in_=WVP[:, 0:2048])
            nc.gpsimd.dma_start(out=wv_sb[:, 2048:4096], in_=WVP[:, 2048:4096])
            xt = [sb.tile([128, S], FP16, tag=f"xt{d}", name=f"xt{d}")
                  for d in range(DINC)]
            w1eng = [nc.sync, nc.gpsimd, nc.scalar, nc.sync,
                     nc.gpsimd, nc.scalar, nc.sync, nc.gpsimd]
            # first key chunk (cols 0:128) alone so V-proj kc=0 starts asap
            for d in range(DINC):
                w1eng[d].dma_start(out=xt[d][:, 0:128], in_=XT[d * 128:(d + 1) * 128, 0:128])
            for d in range(DINC):
                w1eng[d].dma_start(out=xt[d][:, 128:512], in_=XT[d * 128:(d + 1) * 128, 128:512])
            for d in range(DINC):
                w1eng[d].dma_start(out=xt[d][:, 512:1024], in_=XT[d * 128:(d + 1) * 128, 512:1024])
            # pair-0 weights (needed only after the 27us V-proj phase)
            wq_t = [None] * PAIRS
            wk_t = [None] * PAIRS
            wq_t[0] = sb.tile([128, 1024], FP16, tag="wq", bufs=3, name="wq0")
            wk_t[0] = sb.tile([128, 1024], FP16, tag="wk", bufs=3, name="wk0")
            nc.sync.dma_start(out=wq_t[0][:, :], in_=WQP[0, :, :])
            nc.sync.dma_start(out=wk_t[0][:, :], in_=WKP[0, :, :])
            for c0 in range(1024, S, 512):
                for d in range(DINC):
                    w1eng[d].dma_start(out=xt[d][:, c0:c0 + 512],
                                       in_=XT[d * 128:(d + 1) * 128, c0:c0 + 512])
            # output-projection weights, low priority
            wo_sb = sb.tile([128, PAIRS * 1024], FP16, tag="wo", name="wo_sb")
            for j in range(PAIRS):
                nc.gpsimd.dma_start(out=wo_sb[:, j * 1024:(j + 1) * 1024],
                                    in_=WOP[j, :, :])

            # ---- V projection -> augmented V layout, fp16.
            # Per key chunk segment of 772 cols, per pair j at j*193:
            #   [V_h(2j) 64 | ones | ones | junk 63 | V_h(2j+1) 64]
            # psA stationary = cols 0..65 (V_h0|ones): psum row 64 = softmax
            # sums h0.  psB stationary = cols 65..193 (ones|junk|V_h1): psum
            # row 0 = sums h1, rows 64..127 = h1 attention out.
            # Split by pair-half: V01 runs upfront (pair 0 needs it), V23
            # rides the fill queue (needed only from pair 2), so the scalar
            # exp stream starts ~14us earlier.
            vaug = sb.tile([128, KC * VSEG], FP16, tag="vaug", name="vaug")
            vsegs = vaug[:, :].rearrange("p (s c) -> p s c", c=VSEG)
            for j in range(PAIRS):
                nc.vector.memset(vsegs[:, :, j * SEG + 64:j * SEG + 65], 1.0)
                nc.vector.memset(vsegs[:, :, j * SEG + 65:j * SEG + 66], 1.0)

            def gen_vproj(jp, k0=0, k1=KC):
                """V projection for pair-half jp (pairs 2jp, 2jp+1), N=256."""
                for kc in range(k0, k1):
                    vps = ps.tile([128, 256], F32, tag="ps_proj", bufs=2)
                    for d in range(DINC):
                        nc.tensor.matmul(
                            vps[:, :],
                            xt[d][:, kc * 128:(kc + 1) * 128],
                            wv_sb[:, d * 512 + jp * 256:d * 512 + jp * 256 + 256],
                            start=(d == 0), stop=(d == DINC - 1),
                        )
                        if d == 3:
                            yield
                    s0 = kc * VSEG
                    with nc.allow_low_precision(reason="fp16 V"):
                        for jj in range(2):
                            j = 2 * jp + jj
                            o = s0 + j * SEG
                            c = j * 128
                            nc.vector.tensor_add(
                                vaug[:, o:o + 64],
                                vps[:, jj * 128:jj * 128 + 64], bvb[:, c:c + 64])
                            nc.vector.tensor_add(
                                vaug[:, o + 129:o + 193],
                                vps[:, jj * 128 + 64:jj * 128 + 128],
                                bvb[:, c + 64:c + 128])
                    yield

            for _ in gen_vproj(0, 0, 4):
                pass

            qt_pool = [sb.tile([128, S], FP16, tag="qt", bufs=2, name=f"qt{i}") for i in range(2)]
            kt_pool = [sb.tile([128, S], FP16, tag="kt", bufs=2, name=f"kt{i}") for i in range(2)]
            aot = [sb.tile([128, S], FP16, tag=f"ao{j}", name=f"ao{j}")
                   for j in range(PAIRS)]

            def gen_proj(wt, out_t, bias_col, t0=0, t1=QT):
                """Q/K projection tiles [t0,t1) as a generator of small
                PE pieces (2 matmuls each) for interleaving."""
                for tt in range(t0, t1):
                    pp = ps.tile([128, 512], F32, tag="ps_proj", bufs=2)
                    for d0 in range(0, DINC, 2):
                        for d in (d0, d0 + 1):
                            nc.tensor.matmul(
                                pp[:, :],
                                wt[:, d * 128:(d + 1) * 128],
                                xt[d][:, tt * 512:(tt + 1) * 512],
                                start=(d == 0), stop=(d == DINC - 1),
                            )
                        yield
                    with nc.allow_low_precision(reason="fp16 qk"):
                        nc.vector.tensor_scalar_add(
                            out_t[:, tt * 512:(tt + 1) * 512], pp[:, :],
                            bqk[:, bias_col:bias_col + 1],
                        )

            y_tiles = {}

            def gen_oproj(c0, c1):
                """Output-projection token chunks [c0, c1) as PE pieces."""
                for c in range(c0, c1):
                    ysb = sb.tile([128, 1024], BF16, tag="y", bufs=2)
                    y_tiles[c] = ysb
                    for nt in range(2):
                        yps = ps.tile([128, 512], F32, tag="ps_proj", bufs=2)
                        for jj in range(PAIRS):
                            nc.tensor.matmul(
                                yps[:, :],
                                aot[jj][:, c * 128:(c + 1) * 128],
                                wo_sb[:, jj * 1024 + nt * 512: jj * 1024 + nt * 512 + 512],
                                start=(jj == 0), stop=(jj == PAIRS - 1),
                            )
                        with nc.allow_low_precision(reason="bf16 partial out"):
                            nc.vector.tensor_copy(
                                ysb[:, nt * 512:(nt + 1) * 512], yps[:, :])
                        yield
                    nc.gpsimd.dma_start(
                        out=Y[c * 128:(c + 1) * 128, :], in_=ysb[:, :])
                    yield

            # fill machinery: a list of (generator) producers pumped one piece
            # at a time inside the attention loop; closures (tail part B) take
            # priority.
            import collections
            fq = collections.deque()

            def pump():
                while fq:
                    item = fq[0]
                    if callable(item):
                        fq.popleft()
                        item()
                        return
                    try:
                        next(item)
                        return
                    except StopIteration:
                        fq.popleft()
                        continue

            def drain():
                while fq:
                    pump()

            def make_tail(j, qsl, psA, psB):
                """Softmax tail for one (pair, q2).  Part A (inline): stage
                the PV psums to SBUF.  Slot pieces dispatched in the NEXT q2:
                1/den = exp(-ln(den)) on the [1,512] sums rows only (~0.7us
                scalar pieces; ln/exp share one activation table so no table
                reloads), then per head a fp16 ones-matmul broadcast into a
                short-lived ps_proj tile + DVE multiply straight from PSUM.
                """
                # h1's sums are copied into cols 0:512 (same as h0, on
                # partition 0) so the ln/exp span is [65,512], not [65,1024]
                srow = sb.tile([65, 512], F32, tag="srow", bufs=4)
                rrow = sb.tile([65, 512], FP16, tag="rrow", bufs=4)
                aocp = sb.tile([128, 1024], F32, tag="aocp", bufs=4)
                nc.vector.tensor_copy(srow[64:65, 0:512], psA[64:65, :])
                nc.vector.tensor_copy(srow[0:1, 0:512], psB[0:1, :])
                nc.vector.tensor_copy(aocp[0:64, 0:512], psA[0:64, :])
                nc.vector.tensor_copy(aocp[64:128, 512:1024], psB[64:128, :])

                lrow = sb.tile([65, 512], F32, tag="lrow", bufs=4)

                def t_ln():
                    # 1/den = exp(-ln(den)): ln/exp share one activation
                    # table (no reloads); one [65,512] span covers both
                    # heads' sums rows (junk lanes harmlessly processed).
                    # DVE reciprocal is NOT used: at 6.5ns/free-elem the
                    # [1,512] rows cost 3.3us each and saturate the DVE.
                    nc.scalar.activation(lrow[0:65, :], srow[0:65, :],
                                         mybir.ActivationFunctionType.Ln)

                def t_exp():
                    nc.scalar.activation(rrow[0:65, :], lrow[0:65, :],
                                         mybir.ActivationFunctionType.Exp,
                                         scale=-1.0)

                def t_h0():
                    psbc0 = ps.tile([128, 512], F32, tag="ps_proj", bufs=2)
                    nc.tensor.matmul(psbc0[:, :], ones16[64:65, :],
                                     rrow[64:65, 0:512],
                                     start=True, stop=True)
                    with nc.allow_low_precision(reason="fp16 out"):
                        nc.vector.tensor_mul(
                            aot[j][0:64, qsl], aocp[0:64, 0:512], psbc0[0:64, :])

                def t_h1():
                    psbc1 = ps.tile([128, 512], F32, tag="ps_proj", bufs=2)
                    nc.tensor.matmul(psbc1[:, :], ones16[0:1, :],
                                     rrow[0:1, 0:512],
                                     start=True, stop=True)
                    with nc.allow_low_precision(reason="fp16 out"):
                        nc.vector.tensor_mul(
                            aot[j][64:128, qsl], aocp[64:128, 512:1024],
                            psbc1[64:128, :])

                return {0: t_ln, 2: t_exp, 9: t_h0, 11: t_h1}

            # ---- upfront: K0 fully (scores q2=0 need all key tiles) and
            # only the first Q0 tile; Q0 tiles 1-3 and the V23 projection
            # join the fill queue with loose deadlines.
            qt_cur, kt_cur = qt_pool[0], kt_pool[0]
            for _ in gen_proj(wk_t[0], kt_cur, 1):
                pass
            for _ in gen_proj(wq_t[0], qt_cur, 0, 0, 1):
                pass
            # V01 kc>=4 arrives JIT through the double-pumped q2=0 fill
            # (2 pieces/kc keeps it 4 key chunks ahead of the PV consumer);
            # Q0 tiles 1-3 follow before q2=1 needs them.
            fq.append(gen_vproj(0, 4, KC))
            fq.append(gen_proj(wq_t[0], qt_cur, 0, 1, QT))
            fq.append(gen_vproj(1, 0, 8))

            # ---- main loop over head pairs
            tail_pieces = None   # pair 3: pending tail of the previous q2
            pending_tails = []   # pairs 0-2: whole tails for the pair-end drain
            for j in range(PAIRS):
                if j < PAIRS - 1:
                    wq_t[j + 1] = sb.tile([128, 1024], FP16, tag="wq", bufs=3, name=f"wq{j+1}")
                    wk_t[j + 1] = sb.tile([128, 1024], FP16, tag="wk", bufs=3, name=f"wk{j+1}")
                    nc.sync.dma_start(out=wq_t[j + 1][:, :], in_=WQP[j + 1, :, :])
                    nc.sync.dma_start(out=wk_t[j + 1][:, :], in_=WKP[j + 1, :, :])
                    qt_nxt = qt_pool[(j + 1) % 2]
                    kt_nxt = kt_pool[(j + 1) % 2]
                    fq.append(gen_proj(wq_t[j + 1], qt_nxt, 2 * (j + 1)))
                    fq.append(gen_proj(wk_t[j + 1], kt_nxt, 2 * (j + 1) + 1))
                    if j == 0:
                        fq.append(gen_vproj(1, 8, KC))

                for q2 in range(QT):
                    if j == PAIRS - 1 and q2 == 2:
                        # O-proj chunks 0..7 ride along pair 3's q2=2..3;
                        # their aot[3] slices (q2 0..1) are complete by then
                        fq.append(gen_oproj(0, 8))
                    qsl = slice(q2 * 512, (q2 + 1) * 512)
                    psA = ps.tile([65, 512], F32, tag="ps_pv", bufs=2)
                    psB = ps.tile([128, 512], F32, tag="ps_pv", bufs=2)
                    for kc in range(KC):
                        pss = ps.tile([128, 1024], F32, tag="ps_s", bufs=2)
                        ksl = slice(kc * 128, (kc + 1) * 128)
                        nc.tensor.matmul(
                            pss[:, 0:512], kt_cur[0:64, ksl], qt_cur[0:64, qsl],
                            start=True, stop=True,
                        )
                        nc.tensor.matmul(
                            pss[:, 512:1024], kt_cur[64:128, ksl], qt_cur[64:128, qsl],
                            start=True, stop=True,
                        )
                        pt = sb.tile([128, 1024], BF16, tag="pt", bufs=8)
                        nc.scalar.activation(
                            pt[:, :], pss[:, :],
                            mybir.ActivationFunctionType.Exp,
                        )
                        s0 = kc * VSEG + j * SEG
                        nc.tensor.matmul(
                            psA[:, :], vaug[:, s0:s0 + 65], pt[:, 0:512],
                            start=(kc == 0), stop=(kc == KC - 1),
                        )
                        nc.tensor.matmul(
                            psB[:, :], vaug[:, s0 + 65:s0 + 193], pt[:, 512:1024],
                            start=(kc == 0), stop=(kc == KC - 1),
                        )
                        # pair 3 only: previous q2's tail pieces at fixed
                        # slots (timed so the PE bcast never waits on the
                        # scalar ln/exp).  Pairs 0-2 defer whole tails to the
                        # pair-end drain block (their aot is read only in
                        # pair 3), keeping the scalar stream pure exps.
                        # NOTE: FULL matmul fill here raises sustained chip
                        # power and the package DVFS cuts the clock ~20%, a
                        # net loss (measured); the 8-slot fill density stays
                        # under the knee.
                        if tail_pieces:
                            fn = tail_pieces.pop(kc, None)
                            if fn is not None:
                                fn()
                                if not tail_pieces:
                                    tail_pieces = None
                        if j == 0 and q2 == 0:
                            pump()
                            pump()
                        elif kc in (1, 3, 4, 5, 6, 7, 13, 15) or (
                                j <= 1 and kc in (12, 14)) or (
                                j == PAIRS - 1 and kc in (8, 12, 14)):
                            pump()

                    new_tail = make_tail(j, qsl, psA, psB)
                    if j < PAIRS - 1:
                        pending_tails.append(new_tail)
                    elif q2 == QT - 1:
                        final_tail = new_tail
                    else:
                        assert tail_pieces is None
                        tail_pieces = new_tail

                # phased: next pair's projections (and any queued O-proj
                # chunks) run as a block here; the deferred tails' scalar
                # ln/exp runs under this block (the scalar is idle here),
                # then their broadcasts/muls follow the drained PE work.
                for t in pending_tails:
                    t[0]()
                    t[2]()
                drain()
                for t in pending_tails:
                    t[9]()
                    t[11]()
                pending_tails = []
                if j < PAIRS - 1:
                    qt_cur, kt_cur = qt_nxt, kt_nxt

            # ---- endgame: start the final tail's scalar chain at once, then
            # emit O-proj chunks that don't need the last q2 while it runs,
            # then the final broadcasts/muls, then the last chunks.
            final_tail[0]()
            final_tail[2]()
            for _ in gen_oproj(8, 12):
                pass
            final_tail[9]()
            final_tail[11]()
            for _ in gen_oproj(12, 16):
                pass

    _split_multi_waits(nc)
    return nc


_nc_cache = {}
_last_results = None


def _get_nc():
    if "nc" not in _nc_cache:
        _nc_cache["nc"] = build_bass()
    return _nc_cache["nc"]


def _prep_weights(hh, wq, bq, wk, bk, wv, bv, wo):
    """Pack the head-half hh slice (heads hh*8..hh*8+8) of all weights."""
    sl = slice(hh * 512, (hh + 1) * 512)
    scale = np.float32(1.0 / np.sqrt(DH))
    wqT = np.ascontiguousarray(wq.T[:, sl]) * scale   # [1024, 512]
    wkT = np.ascontiguousarray(wk.T[:, sl])
    wvT = np.ascontiguousarray(wv.T[:, sl])
    woT = np.ascontiguousarray(wo.T[sl, :])           # [512, 1024]
    # WQP[j, p, (d m)] = wqT[d*128+p, j*128+m]
    A = wqT.reshape(DINC, 128, PAIRS, 128)
    WQP = np.ascontiguousarray(A.transpose(2, 1, 0, 3).reshape(PAIRS, 128, 1024)).astype(np.float16)
    A = wkT.reshape(DINC, 128, PAIRS, 128)
    WKP = np.ascontiguousarray(A.transpose(2, 1, 0, 3).reshape(PAIRS, 128, 1024)).astype(np.float16)
    # WVP[p, (d n)] = wvT[d*128+p, n]
    A = wvT.reshape(DINC, 128, 512)
    WVP = np.ascontiguousarray(A.transpose(1, 0, 2).reshape(128, 4096)).astype(np.float16)
    # WOP[j, p, n] = woT[j*128+p, n]
    WOP = np.ascontiguousarray(woT.reshape(PAIRS, 128, 1024)).astype(np.float16)
    bqs = (bq[sl] * scale).reshape(PAIRS, 128)
    bkr = bk[sl].reshape(PAIRS, 128)
    BQK = np.empty((128, 2 * PAIRS), np.float32)
    for jx in range(PAIRS):
        BQK[:, 2 * jx] = bqs[jx]
        BQK[:, 2 * jx + 1] = bkr[jx]
    BVB = np.ascontiguousarray(np.tile(bv[sl].reshape(1, 512), (128, 1)))
    return {"WQP": WQP, "WKP": WKP, "WVP": WVP, "WOP": WOP,
            "BQK": BQK, "BVB": BVB}


def kernel(x_input, wq, bq, wk, bk, wv, bv, wo, bo):
    x_input = np.asarray(x_input, dtype=np.float32)
    wq, bq = np.asarray(wq, np.float32), np.asarray(bq, np.float32)
    wk, bk = np.asarray(wk, np.float32), np.asarray(bk, np.float32)
    wv, bv = np.asarray(wv, np.float32), np.asarray(bv, np.float32)
    wo, bo = np.asarray(wo, np.float32), np.asarray(bo, np.float32)

    wsets = [_prep_weights(hh, wq, bq, wk, bk, wv, bv, wo) for hh in range(2)]
    xTs = [np.ascontiguousarray(x_input[b].T).astype(np.float16) for b in range(B)]

    nc = _get_nc()
    in_maps = []
    for c in range(N_CORES):
        m = dict(wsets[c % 2])
        m["XT"] = xTs[c // 2]
        in_maps.append(m)

    res = run_bass_kernel_spmd(nc, in_maps, list(range(N_CORES)))
    global _last_results
    _last_results = res

    out = np.empty((B, S, D), np.float32)
    for b in range(B):
        y0 = np.asarray(res.results[2 * b]["Y"]).astype(np.float32)
        y1 = np.asarray(res.results[2 * b + 1]["Y"]).astype(np.float32)
        out[b] = y0 + y1
    out += bo.reshape(1, 1, D)
    return out
